# revision 6
# baseline (speedup 1.0000x reference)
"""ChebyKAN linear layer on 8 Trainium2 NeuronCores.

Math: y[b,j] = sum_{i,k} T_k(tanh(x[b,i])) * C[i,j,k],  k = 0..8.

  - Device computes the PRODUCT basis Q = [T1, T1^2, T1*T2, T2^2, T2*T3,
    T3^2, T3*T4, T4^2] (T2/T3/T4 are transient f32 intermediates). Since
    T_2m = 2*Q_2m - 1 and T_2m+1 = 2*Q_2m+1 - T1, the affine corrections
    fold into the host-side coefficients:
       A_1 = C_1 - C_3 - C_5 - C_7,  A_k = 2*C_k (k>=2),
       bias_j = sum_i (C_0 - C_2 - C_4 - C_6 - C_8)[i,j]
    (bias added during PSUM eviction). Conditioning stays good (~1.7x
    error amplification), so the whole matmul path runs in BF16:
    measured end-to-end rel err ~5e-3 vs the 2e-2 gate.
  - BF16 weights enable Fast-Weight-Load on LDWEIGHTS (f32r got none),
    halve cmat DMA bytes, and keep the PE at its 1 col/cycle stream rate
    (512 matmuls x 512 cols ~= 110us/core floor).
  - ~28 warmup matmuls on a zero tile run during the DMA/prolog head so
    the PE_HAM clock gate reaches 8/8 before the real stream starts.
  - Matmul loop is (ic, k) outer / bc inner: the 4 PSUM banks of a
    superchunk accumulate in an interleaved round-robin, so the stream
    only ever waits on the basis of ONE i-chunk (fast pipeline start).

Sharding: data-parallel over Bv (16384 -> 8 x 2048), cheby_coeffs
replicated (host-relaid-out). Host pre-transposes x (bf16) so the
contraction index i lands on SBUF partitions.
"""

import json as _json

import ml_dtypes
import numpy as np

# ---------------------------------------------------------------------------
# Container workarounds (inlined so kernel.py is self-contained):
#  1. walrus here refuses instructions carrying >1 sem-wait; hoist excess
#     waits onto NoOps inserted before the offender (same engine queue).
#  2. TileContext tail drain accumulates one wait per logical processor;
#     pre-split them the same way.
# ---------------------------------------------------------------------------

import concourse.bass as bass
import concourse.tile as tile
from concourse import mybir
from concourse._compat import with_exitstack
from concourse.bass_utils import run_bass_kernel_spmd
from concourse.vector_clock import ScopedClock, VectorClock

_MAX_WAITS = 1


def _legalize_bir_json(raw: bytes) -> bytes:
    bir = _json.loads(raw)
    changed = False
    for fn in bir.get("functions", []):
        for blk in fn.get("blocks", []):
            out = []
            for inst in blk.get("instructions", []):
                si = inst.get("sync_info")
                waits = (si or {}).get("on_wait") or []
                if len(waits) > _MAX_WAITS:
                    changed = True
                    excess, keep = waits[:-_MAX_WAITS], waits[-_MAX_WAITS:]
                    for j, w in enumerate(excess):
                        out.append(
                            {
                                "debug": inst.get("debug", 0),
                                "engine": inst["engine"],
                                "ins": [],
                                "name": f"{inst['name']}--w{j}",
                                "opcode": "NoOp",
                                "outs": [],
                                "sync_info": {"on_update": [], "on_wait": [w]},
                                "text_hint": "wait_split",
                            }
                        )
                    si["on_wait"] = keep
                out.append(inst)
            blk["instructions"] = out
    return _json.dumps(bir).encode() if changed else raw


def _patched_drain_and_barrier(self, tick_clock, wait_clock):
    gc = tick_clock.global_clock
    n = len(gc)
    for proc in range(n):
        t = gc[proc]
        if t <= 0:
            continue
        vec = [0] * n
        vec[proc] = t
        nop = self.nc.sync.nop(nofuse=True, hint="tail_drain_split")
        wait_clock.add_sem_waits(nop.ins, ScopedClock({None: VectorClock(vec)}))
    self.nc.sync.drain()
    self.nc.all_engine_barrier()
    assert self.sems is not None
    popped = self.nc._tile_sem_poison_stack.pop()
    assert popped is self._sem_poison
    self.nc.clear_and_free_semaphores(list(self.sems.allocated().values()))
    self.nc.all_engine_barrier()


def _apply_patches():
    if getattr(bass.Bass, "_cheby_patched", False):
        return
    orig = bass.Bass.to_json_bytes

    def patched(self, *a, **kw):
        return _legalize_bir_json(orig(self, *a, **kw))

    bass.Bass.to_json_bytes = patched
    tile.TileContext._drain_and_barrier = _patched_drain_and_barrier
    bass.Bass._cheby_patched = True


_apply_patches()

# ---------------------------------------------------------------------------
# Problem constants (hardcoded per the harness contract)
# ---------------------------------------------------------------------------
NCORES = 8
BV, DIM, K = 16384, 512, 9
BC = BV // NCORES          # 2048 rows per core
SC = 512                   # b-superchunk width
NSC = BC // SC             # 4 superchunks per core
NIC = DIM // 128           # 4 i-chunks
NCH = NIC * (K - 1)        # 32 contraction chunks (k = 1..8)
NWARM = 62                 # HAM warmup matmuls (bridge into the real stream)

F32 = mybir.dt.float32
BF16 = mybir.dt.bfloat16
AFT = mybir.ActivationFunctionType
ALU = mybir.AluOpType
BF16_NP = ml_dtypes.bfloat16


def _build_nc():
    nc = bass.Bass()
    # x relaid out on host: [p, s, ic, c] = xT[ic*128+p, s*512+c], bf16
    xh_d = nc.dram_tensor("xh", (128, NSC, NIC, SC), BF16, kind="ExternalInput")
    # cmat[p, ic*8+m, j] = A[ic*128+p, j, m+1], bf16
    cm_d = nc.dram_tensor("cmat", (128, NCH, DIM), BF16, kind="ExternalInput")
    y_d = nc.dram_tensor("y", (BC, DIM), F32, kind="ExternalOutput")

    @with_exitstack
    def kern(ctx, tc):
        nc = tc.nc
        cpool = ctx.enter_context(tc.tile_pool(name="cmat", bufs=4))
        bpool = ctx.enter_context(tc.tile_pool(name="bias", bufs=1))
        xpool = ctx.enter_context(tc.tile_pool(name="x", bufs=5))
        upool = ctx.enter_context(tc.tile_pool(name="u", bufs=14))
        qpool = ctx.enter_context(tc.tile_pool(name="basis", bufs=41))
        ppool = ctx.enter_context(tc.tile_pool(name="ps", bufs=8, space="PSUM"))
        ypool = ctx.enter_context(tc.tile_pool(name="y", bufs=4))

        # ---- HAM warmup: keep the PE busy through the DMA/prolog head so
        # the clock gate is at 8/8 when the real matmuls arrive.
        wtile = qpool.tile([128, 128], BF16, tag="basis", name="warm")
        nc.vector.memset(wtile[:], 0.0)
        negone = bpool.tile([128, 1], F32, tag="negone")
        nc.gpsimd.memset(negone[:], -1.0)
        wps = ppool.tile([128, SC], F32, tag="ps", name="warmps")
        for i in range(NWARM):
            nc.tensor.matmul(
                wps[:, 0:64],
                wtile[:],
                wtile[:, 0:64],
                start=(i == 0),
                stop=(i == NWARM - 1),
                skip_group_check=True,
            )

        # ---- input DMAs, priority-ordered on the sync HWDGE queue:
        # first i-chunk of x (feeds the very first tanh), then the cmat
        # chunks in consumption order, interleaved with the rest of x.
        xs = [None] * NSC  # xs[s]: tile [128, n, SC]
        xs0a = xpool.tile([128, 1, SC], BF16, tag="x", name="x0a")
        nc.sync.dma_start(xs0a[:], xh_d[:, 0, 0:1, :])
        cm01 = cpool.tile([128, 2, DIM], BF16, tag="cmat", name="cm01")
        nc.gpsimd.dma_start(cm01[:], cm_d[:, 0:2, :])
        xs0b = xpool.tile([128, 3, SC], BF16, tag="x", name="x0b")
        nc.sync.dma_start(xs0b[:], xh_d[:, 0, 1:4, :])
        cm27 = cpool.tile([128, 6, DIM], BF16, tag="cmat", name="cm27")
        nc.gpsimd.dma_start(cm27[:], cm_d[:, 2:8, :])
        xs[1] = xpool.tile([128, NIC, SC], BF16, tag="x", name="x1")
        nc.sync.dma_start(xs[1][:], xh_d[:, 1, :, :])
        cm8 = cpool.tile([128, 8, DIM], BF16, tag="cmat", name="cm8")
        nc.sync.dma_start(cm8[:], cm_d[:, 8:16, :])
        xs[2] = xpool.tile([128, NIC, SC], BF16, tag="x", name="x2")
        nc.sync.dma_start(xs[2][:], xh_d[:, 2, :, :])
        cm16 = cpool.tile([128, 16, DIM], BF16, tag="cmat", name="cm16")
        nc.sync.dma_start(cm16[:], cm_d[:, 16:32, :])
        xs[3] = xpool.tile([128, NIC, SC], BF16, tag="x", name="x3")
        nc.sync.dma_start(xs[3][:], xh_d[:, 3, :, :])

        def xview(s, ic):
            if s == 0:
                return xs0a[:, 0, :] if ic == 0 else xs0b[:, ic - 1, :]
            return xs[s][:, ic, :]

        def cmview(c):
            if c < 2:
                return cm01[:, c, :]
            if c < 8:
                return cm27[:, c - 2, :]
            if c < 16:
                return cm8[:, c - 8, :]
            return cm16[:, c - 16, :]

        def basis(s, ic):
            """Emit the 8-product Chebyshev basis for (s, ic); returns the
            bf16 MM tiles Q[0..7]. Engine split: ACT 7 ops (tanh, squares,
            affines), DVE 6 (copies + T1*T2 + fused T3), GPS 2 (products).
            Queue order matches MM consumption order Q0,Q1,...,Q7."""
            xv = xview(s, ic)
            nm = f"{s}_{ic}"
            Q = [
                qpool.tile([128, SC], BF16, tag="basis", name=f"B{nm}_{m}")
                for m in range(8)
            ]
            u = lambda t: upool.tile([128, SC], F32, tag="u", name=f"{t}{nm}")
            T1, Q2f, t2, Q4f, t4, Q3f, t3 = (
                u("T1"), u("Q2f"), u("t2"), u("Q4f"), u("t4"), u("Q3f"), u("t3"),
            )
            nc.scalar.activation(T1[:], xv, AFT.Tanh)
            nc.vector.tensor_copy(Q[0][:], T1[:])
            nc.scalar.activation(Q2f[:], T1[:], AFT.Square)
            nc.vector.tensor_copy(Q[1][:], Q2f[:])
            nc.scalar.activation(t2[:], Q2f[:], AFT.Identity, scale=2.0, bias=negone[:])
            nc.vector.tensor_mul(Q3f[:], T1[:], t2[:])
            nc.vector.tensor_copy(Q[2][:], Q3f[:])
            nc.scalar.activation(Q4f[:], t2[:], AFT.Square)
            # T3 = 2*(T1*T2) - T1, fused on DVE
            nc.vector.scalar_tensor_tensor(
                t3[:], Q3f[:], 2.0, T1[:], ALU.mult, ALU.subtract
            )
            nc.vector.tensor_copy(Q[3][:], Q4f[:])
            nc.scalar.activation(t4[:], Q4f[:], AFT.Identity, scale=2.0, bias=negone[:])
            nc.gpsimd.tensor_mul(Q[4][:], t2[:], t3[:])
            nc.scalar.activation(Q[5][:], t3[:], AFT.Square)
            nc.gpsimd.tensor_mul(Q[6][:], t3[:], t4[:])
            nc.scalar.activation(Q[7][:], t4[:], AFT.Square)
            return Q

        for s in range(NSC):
            Qs = [basis(s, ic) for ic in range(NIC)]
            ps = [
                ppool.tile([128, DIM], F32, tag="ps", name=f"ps{s}_{bc}")
                for bc in range(NSC)
            ]

            def evict(bc):
                yt = ypool.tile([128, DIM], F32, tag="y")
                b0 = s * SC + bc * 128
                if s == NSC - 1 and bc == NSC - 1:
                    # final bank: DVE-only eviction (no scalar-queue latency),
                    # halves DMA'd back-to-back on the sync ring
                    nc.vector.tensor_copy(yt[:, 0:256], ps[bc][:, 0:256])
                    nc.sync.dma_start(y_d[b0 : b0 + 128, 0:256], yt[:, 0:256])
                    nc.vector.tensor_copy(yt[:, 256:512], ps[bc][:, 256:512])
                    nc.sync.dma_start(y_d[b0 : b0 + 128, 256:512], yt[:, 256:512])
                elif bc % 2 == 0:
                    nc.vector.tensor_copy(yt[:], ps[bc][:])
                    nc.sync.dma_start(y_d[b0 : b0 + 128, :], yt[:])
                else:
                    nc.scalar.copy(yt[:], ps[bc][:])
                    nc.scalar.dma_start(y_d[b0 : b0 + 128, :], yt[:])

            if s < NSC - 1:
                # bc-inner: only one i-chunk's basis gates the stream
                for ic in range(NIC):
                    for m in range(8):
                        for bc in range(NSC):
                            nc.tensor.matmul(
                                ps[bc][:],
                                Qs[ic][m][:, bc * 128 : (bc + 1) * 128],
                                cmview(ic * 8 + m),
                                start=(ic == 0 and m == 0),
                                stop=(ic == NIC - 1 and m == 7),
                                skip_group_check=True,
                            )
                for bc in range(NSC):
                    evict(bc)
            else:
                # last superchunk: bc-outer so banks finish progressively
                # and only one eviction+DMA remains after the last matmul
                for bc in range(NSC):
                    for ic in range(NIC):
                        for m in range(8):
                            nc.tensor.matmul(
                                ps[bc][:],
                                Qs[ic][m][:, bc * 128 : (bc + 1) * 128],
                                cmview(ic * 8 + m),
                                start=(ic == 0 and m == 0),
                                stop=(ic == NIC - 1 and m == 7),
                                skip_group_check=True,
                            )
                    evict(bc)

    with tile.TileContext(nc) as tc:
        kern(tc)
    return nc


_NC_CACHE = None


def _get_nc():
    global _NC_CACHE
    if _NC_CACHE is None:
        _NC_CACHE = _build_nc()
    return _NC_CACHE


def _prep_inputs(x, cheby_coeffs):
    C = np.asarray(cheby_coeffs, dtype=np.float64)
    # product-basis coefficient transform (see module docstring)
    A = np.empty((DIM, DIM, K - 1), np.float64)
    A[:, :, 0] = C[:, :, 1] - C[:, :, 3] - C[:, :, 5] - C[:, :, 7]
    for k in range(2, K):
        A[:, :, k - 1] = 2.0 * C[:, :, k]
    bias_j = (
        (C[:, :, 0] - C[:, :, 2] - C[:, :, 4] - C[:, :, 6] - C[:, :, 8])
        .sum(axis=0)
        .astype(np.float32)
    )
    # cmat[p, ic*8+m, j] = A[ic*128+p, j, m+1]
    cmat = np.ascontiguousarray(
        A.astype(np.float32)
        .astype(BF16_NP)
        .reshape(NIC, 128, DIM, K - 1)
        .transpose(1, 0, 3, 2)
        .reshape(128, NCH, DIM)
    )
    xf = np.asarray(x, dtype=np.float32)
    in_maps = []
    for c in range(NCORES):
        xc = xf[c * BC : (c + 1) * BC].astype(BF16_NP)  # (2048, 512)
        # [p, s, ic, cc] = xc[s*512+cc, ic*128+p]
        xh = np.ascontiguousarray(
            xc.reshape(NSC, SC, NIC, 128).transpose(3, 0, 2, 1)
        )
        in_maps.append({"xh": xh, "cmat": cmat})
    return in_maps, bias_j


def kernel(x, cheby_coeffs, _trace=False, _tmpdir=None):
    nc = _get_nc()
    in_maps, bias_j = _prep_inputs(x, cheby_coeffs)
    res = run_bass_kernel_spmd(
        nc,
        in_maps,
        core_ids=list(range(NCORES)),
        trace=_trace,
        tmpdir=_tmpdir,
    )
    y = np.concatenate([r["y"] for r in res.results], axis=0) + bias_j[None, :]
    y = y.astype(np.float32)
    if _trace:
        kernel.last_result = res
    return y


# revision 7
# speedup vs baseline: 1.0195x; 1.0195x over previous
"""ChebyKAN linear layer on 8 Trainium2 NeuronCores.

Math: y[b,j] = sum_{i,k} T_k(tanh(x[b,i])) * C[i,j,k],  k = 0..8.

  - Device computes the PRODUCT basis Q = [T1, T1^2, T1*T2, T2^2, T2*T3,
    T3^2, T3*T4, T4^2] (T2/T3/T4 are transient f32 intermediates). Since
    T_2m = 2*Q_2m - 1 and T_2m+1 = 2*Q_2m+1 - T1, the affine corrections
    fold into the host-side coefficients:
       A_1 = C_1 - C_3 - C_5 - C_7,  A_k = 2*C_k (k>=2),
       bias_j = sum_i (C_0 - C_2 - C_4 - C_6 - C_8)[i,j]
    (bias added during PSUM eviction). Conditioning stays good (~1.7x
    error amplification), so the whole matmul path runs in BF16:
    measured end-to-end rel err ~5e-3 vs the 2e-2 gate.
  - BF16 weights enable Fast-Weight-Load on LDWEIGHTS (f32r got none),
    halve cmat DMA bytes, and keep the PE at its 1 col/cycle stream rate
    (512 matmuls x 512 cols ~= 110us/core floor).
  - ~28 warmup matmuls on a zero tile run during the DMA/prolog head so
    the PE_HAM clock gate reaches 8/8 before the real stream starts.
  - Matmul loop is (ic, k) outer / bc inner: the 4 PSUM banks of a
    superchunk accumulate in an interleaved round-robin, so the stream
    only ever waits on the basis of ONE i-chunk (fast pipeline start).

Sharding: data-parallel over Bv (16384 -> 8 x 2048), cheby_coeffs
replicated (host-relaid-out). Host pre-transposes x (bf16) so the
contraction index i lands on SBUF partitions.
"""

import json as _json

import ml_dtypes
import numpy as np

# ---------------------------------------------------------------------------
# Container workarounds (inlined so kernel.py is self-contained):
#  1. walrus here refuses instructions carrying >1 sem-wait; hoist excess
#     waits onto NoOps inserted before the offender (same engine queue).
#  2. TileContext tail drain accumulates one wait per logical processor;
#     pre-split them the same way.
# ---------------------------------------------------------------------------

import concourse.bass as bass
import concourse.tile as tile
from concourse import mybir
from concourse._compat import with_exitstack
from concourse.bass_utils import run_bass_kernel_spmd
from concourse.vector_clock import ScopedClock, VectorClock

_MAX_WAITS = 1


def _legalize_bir_json(raw: bytes) -> bytes:
    bir = _json.loads(raw)
    changed = False
    for fn in bir.get("functions", []):
        for blk in fn.get("blocks", []):
            out = []
            for inst in blk.get("instructions", []):
                si = inst.get("sync_info")
                waits = (si or {}).get("on_wait") or []
                if len(waits) > _MAX_WAITS:
                    changed = True
                    excess, keep = waits[:-_MAX_WAITS], waits[-_MAX_WAITS:]
                    for j, w in enumerate(excess):
                        out.append(
                            {
                                "debug": inst.get("debug", 0),
                                "engine": inst["engine"],
                                "ins": [],
                                "name": f"{inst['name']}--w{j}",
                                "opcode": "NoOp",
                                "outs": [],
                                "sync_info": {"on_update": [], "on_wait": [w]},
                                "text_hint": "wait_split",
                            }
                        )
                    si["on_wait"] = keep
                out.append(inst)
            blk["instructions"] = out
    return _json.dumps(bir).encode() if changed else raw


def _patched_drain_and_barrier(self, tick_clock, wait_clock):
    gc = tick_clock.global_clock
    n = len(gc)
    for proc in range(n):
        t = gc[proc]
        if t <= 0:
            continue
        vec = [0] * n
        vec[proc] = t
        nop = self.nc.sync.nop(nofuse=True, hint="tail_drain_split")
        wait_clock.add_sem_waits(nop.ins, ScopedClock({None: VectorClock(vec)}))
    self.nc.sync.drain()
    self.nc.all_engine_barrier()
    assert self.sems is not None
    popped = self.nc._tile_sem_poison_stack.pop()
    assert popped is self._sem_poison
    self.nc.clear_and_free_semaphores(list(self.sems.allocated().values()))
    self.nc.all_engine_barrier()


def _apply_patches():
    if getattr(bass.Bass, "_cheby_patched", False):
        return
    orig = bass.Bass.to_json_bytes

    def patched(self, *a, **kw):
        return _legalize_bir_json(orig(self, *a, **kw))

    bass.Bass.to_json_bytes = patched
    tile.TileContext._drain_and_barrier = _patched_drain_and_barrier
    bass.Bass._cheby_patched = True


_apply_patches()

# ---------------------------------------------------------------------------
# Problem constants (hardcoded per the harness contract)
# ---------------------------------------------------------------------------
NCORES = 8
BV, DIM, K = 16384, 512, 9
BC = BV // NCORES          # 2048 rows per core
SC = 512                   # b-superchunk width
NSC = BC // SC             # 4 superchunks per core
NIC = DIM // 128           # 4 i-chunks
NCH = NIC * (K - 1)        # 32 contraction chunks (k = 1..8)
NWARM = 70                 # HAM warmup matmuls (bridge into the real stream)

F32 = mybir.dt.float32
BF16 = mybir.dt.bfloat16
AFT = mybir.ActivationFunctionType
ALU = mybir.AluOpType
BF16_NP = ml_dtypes.bfloat16


def _build_nc():
    nc = bass.Bass()
    # x relaid out on host: [p, s, ic, c] = xT[ic*128+p, s*512+c], bf16
    xh_d = nc.dram_tensor("xh", (128, NSC, NIC, SC), BF16, kind="ExternalInput")
    # cmat[p, ic*8+m, j] = A[ic*128+p, j, m+1], bf16
    cm_d = nc.dram_tensor("cmat", (128, NCH, DIM), BF16, kind="ExternalInput")
    y_d = nc.dram_tensor("y", (BC, DIM), F32, kind="ExternalOutput")

    @with_exitstack
    def kern(ctx, tc):
        nc = tc.nc
        cpool = ctx.enter_context(tc.tile_pool(name="cmat", bufs=4))
        bpool = ctx.enter_context(tc.tile_pool(name="bias", bufs=1))
        xpool = ctx.enter_context(tc.tile_pool(name="x", bufs=5))
        upool = ctx.enter_context(tc.tile_pool(name="u", bufs=14))
        qpool = ctx.enter_context(tc.tile_pool(name="basis", bufs=41))
        ppool = ctx.enter_context(tc.tile_pool(name="ps", bufs=8, space="PSUM"))
        ypool = ctx.enter_context(tc.tile_pool(name="y", bufs=4))

        # ---- HAM warmup: keep the PE busy through the DMA/prolog head so
        # the clock gate is at 8/8 when the real matmuls arrive.
        wtile = qpool.tile([128, 128], BF16, tag="basis", name="warm")
        nc.vector.memset(wtile[:], 0.0)
        negone = bpool.tile([128, 1], F32, tag="negone")
        nc.gpsimd.memset(negone[:], -1.0)
        wps = ppool.tile([128, SC], F32, tag="ps", name="warmps")
        for i in range(NWARM):
            nc.tensor.matmul(
                wps[:, 0:64],
                wtile[:],
                wtile[:, 0:64],
                start=(i == 0),
                stop=(i == NWARM - 1),
                skip_group_check=True,
            )

        # ---- input DMAs, priority-ordered on the sync HWDGE queue:
        # first i-chunk of x (feeds the very first tanh), then the cmat
        # chunks in consumption order, interleaved with the rest of x.
        xs = [None] * NSC  # xs[s]: tile [128, n, SC]
        cm01 = cpool.tile([128, 2, DIM], BF16, tag="cmat", name="cm01")
        nc.scalar.dma_start(cm01[:], cm_d[:, 0:2, :])
        xs0a = xpool.tile([128, 1, SC], BF16, tag="x", name="x0a")
        nc.sync.dma_start(xs0a[:], xh_d[:, 0, 0:1, :])
        xs0b = xpool.tile([128, 3, SC], BF16, tag="x", name="x0b")
        nc.sync.dma_start(xs0b[:], xh_d[:, 0, 1:4, :])
        cm27 = cpool.tile([128, 6, DIM], BF16, tag="cmat", name="cm27")
        nc.sync.dma_start(cm27[:], cm_d[:, 2:8, :])
        xs[1] = xpool.tile([128, NIC, SC], BF16, tag="x", name="x1")
        nc.sync.dma_start(xs[1][:], xh_d[:, 1, :, :])
        cm8 = cpool.tile([128, 8, DIM], BF16, tag="cmat", name="cm8")
        nc.sync.dma_start(cm8[:], cm_d[:, 8:16, :])
        xs[2] = xpool.tile([128, NIC, SC], BF16, tag="x", name="x2")
        nc.sync.dma_start(xs[2][:], xh_d[:, 2, :, :])
        cm16 = cpool.tile([128, 16, DIM], BF16, tag="cmat", name="cm16")
        nc.sync.dma_start(cm16[:], cm_d[:, 16:32, :])
        xs[3] = xpool.tile([128, NIC, SC], BF16, tag="x", name="x3")
        nc.sync.dma_start(xs[3][:], xh_d[:, 3, :, :])

        def xview(s, ic):
            if s == 0:
                return xs0a[:, 0, :] if ic == 0 else xs0b[:, ic - 1, :]
            return xs[s][:, ic, :]

        def cmview(c):
            if c < 2:
                return cm01[:, c, :]
            if c < 8:
                return cm27[:, c - 2, :]
            if c < 16:
                return cm8[:, c - 8, :]
            return cm16[:, c - 16, :]

        def basis(s, ic):
            """Emit the 8-product Chebyshev basis for (s, ic); returns the
            bf16 MM tiles Q[0..7]. Engine split: ACT 7 ops (tanh, squares,
            affines), DVE 6 (copies + T1*T2 + fused T3), GPS 2 (products).
            Queue order matches MM consumption order Q0,Q1,...,Q7."""
            xv = xview(s, ic)
            nm = f"{s}_{ic}"
            Q = [
                qpool.tile([128, SC], BF16, tag="basis", name=f"B{nm}_{m}")
                for m in range(8)
            ]
            u = lambda t: upool.tile([128, SC], F32, tag="u", name=f"{t}{nm}")
            T1, Q2f, t2, Q4f, t4, Q3f, t3 = (
                u("T1"), u("Q2f"), u("t2"), u("Q4f"), u("t4"), u("Q3f"), u("t3"),
            )
            nc.scalar.activation(T1[:], xv, AFT.Tanh)
            nc.vector.tensor_copy(Q[0][:], T1[:])
            nc.scalar.activation(Q2f[:], T1[:], AFT.Square)
            nc.vector.tensor_copy(Q[1][:], Q2f[:])
            nc.scalar.activation(t2[:], Q2f[:], AFT.Identity, scale=2.0, bias=negone[:])
            nc.vector.tensor_mul(Q3f[:], T1[:], t2[:])
            nc.vector.tensor_copy(Q[2][:], Q3f[:])
            nc.scalar.activation(Q4f[:], t2[:], AFT.Square)
            # T3 = 2*(T1*T2) - T1, fused on DVE
            nc.vector.scalar_tensor_tensor(
                t3[:], Q3f[:], 2.0, T1[:], ALU.mult, ALU.subtract
            )
            nc.vector.tensor_copy(Q[3][:], Q4f[:])
            nc.scalar.activation(t4[:], Q4f[:], AFT.Identity, scale=2.0, bias=negone[:])
            nc.gpsimd.tensor_mul(Q[4][:], t2[:], t3[:])
            nc.scalar.activation(Q[5][:], t3[:], AFT.Square)
            nc.gpsimd.tensor_mul(Q[6][:], t3[:], t4[:])
            nc.scalar.activation(Q[7][:], t4[:], AFT.Square)
            return Q

        for s in range(NSC):
            Qs = [basis(s, ic) for ic in range(NIC)]
            ps = [
                ppool.tile([128, DIM], F32, tag="ps", name=f"ps{s}_{bc}")
                for bc in range(NSC)
            ]

            def evict(bc):
                yt = ypool.tile([128, DIM], F32, tag="y")
                b0 = s * SC + bc * 128
                if bc % 2 == 0 or bc == NSC - 1:
                    nc.vector.tensor_copy(yt[:], ps[bc][:])
                    nc.sync.dma_start(y_d[b0 : b0 + 128, :], yt[:])
                else:
                    nc.scalar.copy(yt[:], ps[bc][:])
                    nc.scalar.dma_start(y_d[b0 : b0 + 128, :], yt[:])

            if s < NSC - 1:
                # bc-inner: only one i-chunk's basis gates the stream
                for ic in range(NIC):
                    for m in range(8):
                        for bc in range(NSC):
                            nc.tensor.matmul(
                                ps[bc][:],
                                Qs[ic][m][:, bc * 128 : (bc + 1) * 128],
                                cmview(ic * 8 + m),
                                start=(ic == 0 and m == 0),
                                stop=(ic == NIC - 1 and m == 7),
                                skip_group_check=True,
                            )
                for bc in range(NSC):
                    evict(bc)
            else:
                # last superchunk: bc-outer so banks finish progressively
                # and only one eviction+DMA remains after the last matmul
                for bc in range(NSC):
                    for ic in range(NIC):
                        for m in range(8):
                            nc.tensor.matmul(
                                ps[bc][:],
                                Qs[ic][m][:, bc * 128 : (bc + 1) * 128],
                                cmview(ic * 8 + m),
                                start=(ic == 0 and m == 0),
                                stop=(ic == NIC - 1 and m == 7),
                                skip_group_check=True,
                            )
                    evict(bc)

    with tile.TileContext(nc) as tc:
        kern(tc)
    return nc


_NC_CACHE = None


def _get_nc():
    global _NC_CACHE
    if _NC_CACHE is None:
        _NC_CACHE = _build_nc()
    return _NC_CACHE


def _prep_inputs(x, cheby_coeffs):
    C = np.asarray(cheby_coeffs, dtype=np.float64)
    # product-basis coefficient transform (see module docstring)
    A = np.empty((DIM, DIM, K - 1), np.float64)
    A[:, :, 0] = C[:, :, 1] - C[:, :, 3] - C[:, :, 5] - C[:, :, 7]
    for k in range(2, K):
        A[:, :, k - 1] = 2.0 * C[:, :, k]
    bias_j = (
        (C[:, :, 0] - C[:, :, 2] - C[:, :, 4] - C[:, :, 6] - C[:, :, 8])
        .sum(axis=0)
        .astype(np.float32)
    )
    # cmat[p, ic*8+m, j] = A[ic*128+p, j, m+1]
    cmat = np.ascontiguousarray(
        A.astype(np.float32)
        .astype(BF16_NP)
        .reshape(NIC, 128, DIM, K - 1)
        .transpose(1, 0, 3, 2)
        .reshape(128, NCH, DIM)
    )
    xf = np.asarray(x, dtype=np.float32)
    in_maps = []
    for c in range(NCORES):
        xc = xf[c * BC : (c + 1) * BC].astype(BF16_NP)  # (2048, 512)
        # [p, s, ic, cc] = xc[s*512+cc, ic*128+p]
        xh = np.ascontiguousarray(
            xc.reshape(NSC, SC, NIC, 128).transpose(3, 0, 2, 1)
        )
        in_maps.append({"xh": xh, "cmat": cmat})
    return in_maps, bias_j


def kernel(x, cheby_coeffs, _trace=False, _tmpdir=None):
    nc = _get_nc()
    in_maps, bias_j = _prep_inputs(x, cheby_coeffs)
    res = run_bass_kernel_spmd(
        nc,
        in_maps,
        core_ids=list(range(NCORES)),
        trace=_trace,
        tmpdir=_tmpdir,
    )
    y = np.concatenate([r["y"] for r in res.results], axis=0) + bias_j[None, :]
    y = y.astype(np.float32)
    if _trace:
        kernel.last_result = res
    return y


# revision 8
# speedup vs baseline: 1.0298x; 1.0101x over previous
"""ChebyKAN linear layer on 8 Trainium2 NeuronCores.

Math: y[b,j] = sum_{i,k} T_k(tanh(x[b,i])) * C[i,j,k],  k = 0..8.

  - Device computes the PRODUCT basis Q = [T1, T1^2, T1*T2, T2^2, T2*T3,
    T3^2, T3*T4, T4^2] (T2/T3/T4 are transient f32 intermediates). Since
    T_2m = 2*Q_2m - 1 and T_2m+1 = 2*Q_2m+1 - T1, the affine corrections
    fold into the host-side coefficients:
       A_1 = C_1 - C_3 - C_5 - C_7,  A_k = 2*C_k (k>=2),
       bias_j = sum_i (C_0 - C_2 - C_4 - C_6 - C_8)[i,j]
    (bias added during PSUM eviction). Conditioning stays good (~1.7x
    error amplification), so the whole matmul path runs in BF16:
    measured end-to-end rel err ~5e-3 vs the 2e-2 gate.
  - BF16 weights enable Fast-Weight-Load on LDWEIGHTS (f32r got none),
    halve cmat DMA bytes, and keep the PE at its 1 col/cycle stream rate
    (512 matmuls x 512 cols ~= 110us/core floor).
  - ~28 warmup matmuls on a zero tile run during the DMA/prolog head so
    the PE_HAM clock gate reaches 8/8 before the real stream starts.
  - Matmul loop is (ic, k) outer / bc inner: the 4 PSUM banks of a
    superchunk accumulate in an interleaved round-robin, so the stream
    only ever waits on the basis of ONE i-chunk (fast pipeline start).

Sharding: data-parallel over Bv (16384 -> 8 x 2048), cheby_coeffs
replicated (host-relaid-out). Host pre-transposes x (bf16) so the
contraction index i lands on SBUF partitions.
"""

import json as _json

import ml_dtypes
import numpy as np

# ---------------------------------------------------------------------------
# Container workarounds (inlined so kernel.py is self-contained):
#  1. walrus here refuses instructions carrying >1 sem-wait; hoist excess
#     waits onto NoOps inserted before the offender (same engine queue).
#  2. TileContext tail drain accumulates one wait per logical processor;
#     pre-split them the same way.
# ---------------------------------------------------------------------------

import concourse.bass as bass
import concourse.tile as tile
from concourse import mybir
from concourse._compat import with_exitstack
from concourse.bass_utils import run_bass_kernel_spmd
from concourse.vector_clock import ScopedClock, VectorClock

_MAX_WAITS = 1


def _legalize_bir_json(raw: bytes) -> bytes:
    bir = _json.loads(raw)
    changed = False
    for fn in bir.get("functions", []):
        for blk in fn.get("blocks", []):
            out = []
            for inst in blk.get("instructions", []):
                si = inst.get("sync_info")
                waits = (si or {}).get("on_wait") or []
                if len(waits) > _MAX_WAITS:
                    changed = True
                    excess, keep = waits[:-_MAX_WAITS], waits[-_MAX_WAITS:]
                    for j, w in enumerate(excess):
                        out.append(
                            {
                                "debug": inst.get("debug", 0),
                                "engine": inst["engine"],
                                "ins": [],
                                "name": f"{inst['name']}--w{j}",
                                "opcode": "NoOp",
                                "outs": [],
                                "sync_info": {"on_update": [], "on_wait": [w]},
                                "text_hint": "wait_split",
                            }
                        )
                    si["on_wait"] = keep
                out.append(inst)
            blk["instructions"] = out
    return _json.dumps(bir).encode() if changed else raw


def _patched_drain_and_barrier(self, tick_clock, wait_clock):
    gc = tick_clock.global_clock
    n = len(gc)
    for proc in range(n):
        t = gc[proc]
        if t <= 0:
            continue
        vec = [0] * n
        vec[proc] = t
        nop = self.nc.sync.nop(nofuse=True, hint="tail_drain_split")
        wait_clock.add_sem_waits(nop.ins, ScopedClock({None: VectorClock(vec)}))
    self.nc.sync.drain()
    self.nc.all_engine_barrier()
    assert self.sems is not None
    popped = self.nc._tile_sem_poison_stack.pop()
    assert popped is self._sem_poison
    self.nc.clear_and_free_semaphores(list(self.sems.allocated().values()))
    self.nc.all_engine_barrier()


def _apply_patches():
    if getattr(bass.Bass, "_cheby_patched", False):
        return
    orig = bass.Bass.to_json_bytes

    def patched(self, *a, **kw):
        return _legalize_bir_json(orig(self, *a, **kw))

    bass.Bass.to_json_bytes = patched
    tile.TileContext._drain_and_barrier = _patched_drain_and_barrier
    bass.Bass._cheby_patched = True


_apply_patches()

# ---------------------------------------------------------------------------
# Problem constants (hardcoded per the harness contract)
# ---------------------------------------------------------------------------
NCORES = 8
BV, DIM, K = 16384, 512, 9
BC = BV // NCORES          # 2048 rows per core
SC = 512                   # b-superchunk width
NSC = BC // SC             # 4 superchunks per core
NIC = DIM // 128           # 4 i-chunks
NCH = NIC * (K - 1)        # 32 contraction chunks (k = 1..8)
NWARM = 76                 # HAM warmup matmuls (bridge into the real stream)

F32 = mybir.dt.float32
BF16 = mybir.dt.bfloat16
AFT = mybir.ActivationFunctionType
ALU = mybir.AluOpType
BF16_NP = ml_dtypes.bfloat16


def _build_nc():
    nc = bass.Bass()
    # x relaid out on host: [p, s, ic, c] = xT[ic*128+p, s*512+c], bf16
    xh_d = nc.dram_tensor("xh", (128, NSC, NIC, SC), BF16, kind="ExternalInput")
    # cmat[p, ic*8+m, j] = A[ic*128+p, j, m+1], bf16
    cm_d = nc.dram_tensor("cmat", (128, NCH, DIM), BF16, kind="ExternalInput")
    y_d = nc.dram_tensor("y", (BC, DIM), F32, kind="ExternalOutput")

    @with_exitstack
    def kern(ctx, tc):
        nc = tc.nc
        cpool = ctx.enter_context(tc.tile_pool(name="cmat", bufs=4))
        bpool = ctx.enter_context(tc.tile_pool(name="bias", bufs=1))
        xpool = ctx.enter_context(tc.tile_pool(name="x", bufs=5))
        upool = ctx.enter_context(tc.tile_pool(name="u", bufs=14))
        qpool = ctx.enter_context(tc.tile_pool(name="basis", bufs=41))
        ppool = ctx.enter_context(tc.tile_pool(name="ps", bufs=8, space="PSUM"))
        ypool = ctx.enter_context(tc.tile_pool(name="y", bufs=4))

        # ---- HAM warmup: keep the PE busy through the DMA/prolog head so
        # the clock gate is at 8/8 when the real matmuls arrive.
        wtile = qpool.tile([128, 128], BF16, tag="basis", name="warm")
        nc.vector.memset(wtile[:], 0.0)
        negone = bpool.tile([128, 1], F32, tag="negone")
        nc.gpsimd.memset(negone[:], -1.0)
        wps = ppool.tile([128, SC], F32, tag="ps", name="warmps")
        for i in range(NWARM):
            nc.tensor.matmul(
                wps[:, 0:64],
                wtile[:],
                wtile[:, 0:64],
                start=(i == 0),
                stop=(i == NWARM - 1),
                skip_group_check=True,
            )

        # ---- input DMAs, priority-ordered on the sync HWDGE queue:
        # first i-chunk of x (feeds the very first tanh), then the cmat
        # chunks in consumption order, interleaved with the rest of x.
        xs = [None] * NSC  # xs[s]: tile [128, n, SC]
        cm01 = cpool.tile([128, 2, DIM], BF16, tag="cmat", name="cm01")
        nc.scalar.dma_start(cm01[:], cm_d[:, 0:2, :])
        xs0a = xpool.tile([128, 1, SC], BF16, tag="x", name="x0a")
        nc.sync.dma_start(xs0a[:], xh_d[:, 0, 0:1, :])
        xs0b = xpool.tile([128, 3, SC], BF16, tag="x", name="x0b")
        nc.sync.dma_start(xs0b[:], xh_d[:, 0, 1:4, :])
        cm27 = cpool.tile([128, 6, DIM], BF16, tag="cmat", name="cm27")
        nc.sync.dma_start(cm27[:], cm_d[:, 2:8, :])
        xs[1] = xpool.tile([128, NIC, SC], BF16, tag="x", name="x1")
        nc.sync.dma_start(xs[1][:], xh_d[:, 1, :, :])
        cm8 = cpool.tile([128, 8, DIM], BF16, tag="cmat", name="cm8")
        nc.sync.dma_start(cm8[:], cm_d[:, 8:16, :])
        xs[2] = xpool.tile([128, NIC, SC], BF16, tag="x", name="x2")
        nc.sync.dma_start(xs[2][:], xh_d[:, 2, :, :])
        cm16 = cpool.tile([128, 16, DIM], BF16, tag="cmat", name="cm16")
        nc.sync.dma_start(cm16[:], cm_d[:, 16:32, :])
        xs[3] = xpool.tile([128, NIC, SC], BF16, tag="x", name="x3")
        nc.sync.dma_start(xs[3][:], xh_d[:, 3, :, :])

        def xview(s, ic):
            if s == 0:
                return xs0a[:, 0, :] if ic == 0 else xs0b[:, ic - 1, :]
            return xs[s][:, ic, :]

        def cmview(c):
            if c < 2:
                return cm01[:, c, :]
            if c < 8:
                return cm27[:, c - 2, :]
            if c < 16:
                return cm8[:, c - 8, :]
            return cm16[:, c - 16, :]

        def basis(s, ic):
            """Emit the 8-product Chebyshev basis for (s, ic); returns the
            bf16 MM tiles Q[0..7]. Engine split: ACT 7 ops (tanh, squares,
            affines), DVE 6 (copies + T1*T2 + fused T3), GPS 2 (products).
            Queue order matches MM consumption order Q0,Q1,...,Q7."""
            xv = xview(s, ic)
            nm = f"{s}_{ic}"
            Q = [
                qpool.tile([128, SC], BF16, tag="basis", name=f"B{nm}_{m}")
                for m in range(8)
            ]
            u = lambda t: upool.tile([128, SC], F32, tag="u", name=f"{t}{nm}")
            T1, Q2f, t2, Q4f, t4, Q3f, t3 = (
                u("T1"), u("Q2f"), u("t2"), u("Q4f"), u("t4"), u("Q3f"), u("t3"),
            )
            nc.scalar.activation(T1[:], xv, AFT.Tanh)
            nc.vector.tensor_copy(Q[0][:], T1[:])
            nc.scalar.activation(Q2f[:], T1[:], AFT.Square)
            nc.vector.tensor_copy(Q[1][:], Q2f[:])
            nc.scalar.activation(t2[:], Q2f[:], AFT.Identity, scale=2.0, bias=negone[:])
            nc.vector.tensor_mul(Q3f[:], T1[:], t2[:])
            nc.vector.tensor_copy(Q[2][:], Q3f[:])
            nc.scalar.activation(Q4f[:], t2[:], AFT.Square)
            # T3 = 2*(T1*T2) - T1, fused on DVE
            nc.vector.scalar_tensor_tensor(
                t3[:], Q3f[:], 2.0, T1[:], ALU.mult, ALU.subtract
            )
            nc.vector.tensor_copy(Q[3][:], Q4f[:])
            nc.scalar.activation(t4[:], Q4f[:], AFT.Identity, scale=2.0, bias=negone[:])
            nc.gpsimd.tensor_mul(Q[4][:], t2[:], t3[:])
            nc.scalar.activation(Q[5][:], t3[:], AFT.Square)
            nc.gpsimd.tensor_mul(Q[6][:], t3[:], t4[:])
            nc.scalar.activation(Q[7][:], t4[:], AFT.Square)
            return Q

        for s in range(NSC):
            Qs = [basis(s, ic) for ic in range(NIC)]
            ps = [
                ppool.tile([128, DIM], F32, tag="ps", name=f"ps{s}_{bc}")
                for bc in range(NSC)
            ]

            def evict(bc):
                yt = ypool.tile([128, DIM], F32, tag="y")
                b0 = s * SC + bc * 128
                if bc % 2 == 0 or bc == NSC - 1:
                    nc.vector.tensor_copy(yt[:], ps[bc][:])
                    nc.sync.dma_start(y_d[b0 : b0 + 128, :], yt[:])
                else:
                    nc.scalar.copy(yt[:], ps[bc][:])
                    nc.scalar.dma_start(y_d[b0 : b0 + 128, :], yt[:])

            # basis-production availability order for the very first i-chunk
            MORD = [0, 1, 2, 3, 5, 4, 7, 6] if s == 0 else list(range(8))
            if s < NSC - 1:
                # bc-inner: only one i-chunk's basis gates the stream
                for ic in range(NIC):
                    for m in (MORD if ic == 0 else range(8)):
                        for bc in range(NSC):
                            nc.tensor.matmul(
                                ps[bc][:],
                                Qs[ic][m][:, bc * 128 : (bc + 1) * 128],
                                cmview(ic * 8 + m),
                                start=(ic == 0 and m == MORD[0]),
                                stop=(ic == NIC - 1 and m == 7),
                                skip_group_check=True,
                            )
                for bc in range(NSC):
                    evict(bc)
            else:
                # last superchunk: bc-outer so banks finish progressively;
                # the final bank accumulates in two j-half groups so only a
                # half-width eviction+DMA trails the very last matmul
                for bc in range(NSC):
                    if bc < NSC - 1:
                        for ic in range(NIC):
                            for m in range(8):
                                nc.tensor.matmul(
                                    ps[bc][:],
                                    Qs[ic][m][:, bc * 128 : (bc + 1) * 128],
                                    cmview(ic * 8 + m),
                                    start=(ic == 0 and m == 0),
                                    stop=(ic == NIC - 1 and m == 7),
                                    skip_group_check=True,
                                )
                        evict(bc)
                    else:
                        b0 = s * SC + bc * 128
                        for h in range(2):
                            j0, j1 = h * 256, (h + 1) * 256
                            for ic in range(NIC):
                                for m in range(8):
                                    nc.tensor.matmul(
                                        ps[bc][:, j0:j1],
                                        Qs[ic][m][:, bc * 128 : (bc + 1) * 128],
                                        cmview(ic * 8 + m)[:, j0:j1],
                                        start=(ic == 0 and m == 0),
                                        stop=(ic == NIC - 1 and m == 7),
                                        skip_group_check=True,
                                    )
                            yt = ypool.tile([128, 256], F32, tag="y")
                            nc.vector.tensor_copy(yt[:], ps[bc][:, j0:j1])
                            nc.sync.dma_start(y_d[b0 : b0 + 128, j0:j1], yt[:])

    with tile.TileContext(nc) as tc:
        kern(tc)
    return nc


_NC_CACHE = None


def _get_nc():
    global _NC_CACHE
    if _NC_CACHE is None:
        _NC_CACHE = _build_nc()
    return _NC_CACHE


def _prep_inputs(x, cheby_coeffs):
    C = np.asarray(cheby_coeffs, dtype=np.float64)
    # product-basis coefficient transform (see module docstring)
    A = np.empty((DIM, DIM, K - 1), np.float64)
    A[:, :, 0] = C[:, :, 1] - C[:, :, 3] - C[:, :, 5] - C[:, :, 7]
    for k in range(2, K):
        A[:, :, k - 1] = 2.0 * C[:, :, k]
    bias_j = (
        (C[:, :, 0] - C[:, :, 2] - C[:, :, 4] - C[:, :, 6] - C[:, :, 8])
        .sum(axis=0)
        .astype(np.float32)
    )
    # cmat[p, ic*8+m, j] = A[ic*128+p, j, m+1]
    cmat = np.ascontiguousarray(
        A.astype(np.float32)
        .astype(BF16_NP)
        .reshape(NIC, 128, DIM, K - 1)
        .transpose(1, 0, 3, 2)
        .reshape(128, NCH, DIM)
    )
    xf = np.asarray(x, dtype=np.float32)
    in_maps = []
    for c in range(NCORES):
        xc = xf[c * BC : (c + 1) * BC].astype(BF16_NP)  # (2048, 512)
        # [p, s, ic, cc] = xc[s*512+cc, ic*128+p]
        xh = np.ascontiguousarray(
            xc.reshape(NSC, SC, NIC, 128).transpose(3, 0, 2, 1)
        )
        in_maps.append({"xh": xh, "cmat": cmat})
    return in_maps, bias_j


def kernel(x, cheby_coeffs, _trace=False, _tmpdir=None):
    nc = _get_nc()
    in_maps, bias_j = _prep_inputs(x, cheby_coeffs)
    res = run_bass_kernel_spmd(
        nc,
        in_maps,
        core_ids=list(range(NCORES)),
        trace=_trace,
        tmpdir=_tmpdir,
    )
    y = np.concatenate([r["y"] for r in res.results], axis=0) + bias_j[None, :]
    y = y.astype(np.float32)
    if _trace:
        kernel.last_result = res
    return y
